# revision 1
# baseline (speedup 1.0000x reference)
"""Trainium2 Bass kernel for NeuralTensorLayer (order-1/2/3 polynomial layer).

    out[b,l] = bias[l] + sum_i X[b,i] W1[i,l]
             + sum_ij X[b,i] X[b,j] W2[i,j,l]
             + sum_ijk X[b,i] X[b,j] X[b,k] W3[i,j,k,l]

with B=32768, D=K=32, data-parallel over 8 NeuronCores (4096 rows each).

Strategy (per core):
  * Exploit (i,j) symmetry: only the 528 pairs i<=j are needed against
    host-symmetrized weights W3s[ij,k,l] = W3[i,j,k,l]+W3[j,i,k,l] (i<j),
    cutting the dominant matmul contraction from 1024 -> 528 (+32 X rows).
  * The pair operands arrive pre-gathered from the host in transposed
    layout (XE[p,b]=X[b,i_p], XR[p,b]=X[b,j_p], bf16); the DVE multiplies
    them at 2x into Z^T[p,b] = X_i X_j (bf16).  Contraction chunks:
    4x128 pairs + a 48-row chunk holding 16 pairs plus X^T itself (the
    order-1 rows, DMA'd straight from the host-pretransposed X^T).
  * One fused matmul group (bf16, fp32 PSUM accumulation) per 128-row tile:
    T[b, l*34+k] = sum_p Z^T[p,b] Wcat[p, l*34+k], where k<32 are the
    order-3 T3 columns, k=32 is out_low (W2s rows + W1 on the X rows), and
    k=33 is zero padding (keeps the DVE post-multiply 4B-aligned at 2x).
  * Post: stage T to SBUF bf16 (ScalarE), U = T * Xext broadcast (DVE 2x,
    Xext host-padded with [1,0] cols), reduce over k=34 -> out (DVE).
    bias added on host.
"""

import numpy as np
import ml_dtypes
from contextlib import ExitStack

import concourse.bass as bass
import concourse.bacc as bacc
import concourse.tile as tile
from concourse import mybir
from concourse import bass_utils

BF16 = ml_dtypes.bfloat16

B, D, KOUT = 32768, 32, 32
NCORES = 8
BLOC = B // NCORES          # 4096 rows per core
P = 128                     # rows per tile
SUPER = 4                   # tiles per supertile
SP = SUPER * P              # 512
NSUPER = BLOC // SP         # 8
NPAIRS = D * (D + 1) // 2   # 528
CHUNKS = [128, 128, 128, 128, 16]   # pair rows per contraction chunk
CHUNK_P = [128, 128, 128, 128, 48]  # partitions per chunk (chunk4: +32 X rows)
KG = 34                     # k-grid width: 32 order-3 + out_low + zero pad
NCOL = KOUT * KG            # 1088 psum columns
XW = D + 2                  # host-padded X width: 32 + [1.0, 0.0]

PAIRS = [(i, j) for i in range(D) for j in range(i, D)]
I_P = np.array([p[0] for p in PAIRS], np.int32)
J_P = np.array([p[1] for p in PAIRS], np.int32)

F32 = mybir.dt.float32
BF = mybir.dt.bfloat16


# Drop redundant LDWEIGHTS from the BIR before walrus codegen: matmuls that
# share a stationary operand (the three N-splits per contraction chunk)
# each carry their own Ldweights (walrus's ldw-opt pass is disabled/broken).
# A load is elided when the previous PE weight-op in SCHEDULED order has a
# byte-identical weight AP and the load itself carries no semaphore
# waits/updates (so the PE weight registers provably still hold the same
# data and no sync edge is lost).
def _dedup_ldweights(bir_json: bytes) -> bytes:
    import json as _json

    d = _json.loads(bir_json)
    for fn in d.get("functions", []):
        for blk in fn.get("blocks", []):
            out = []
            last = None
            for i in blk.get("instructions", []):
                if i.get("engine") == "PE" and i.get("opcode") in ("Ldweights", "Matmult"):
                    w = i["ins"][-1] if i["opcode"] == "Matmult" else i["ins"][0]
                    key = (w.get("memref"), w.get("offset"), _json.dumps(w.get("ap")),
                           w.get("dtype"), _json.dumps(i.get("tile_position")),
                           _json.dumps(i.get("tile_size")), i.get("perf_mode"))
                    if i["opcode"] == "Ldweights":
                        si = i.get("sync_info") or {}
                        if (key == last and not si.get("on_wait")
                                and not si.get("on_update")):
                            continue
                        last = key
                    else:
                        # a Matmult's weight ref mirrors the loaded state
                        # (self-loading or not), so it may refresh `last`
                        last = key
                elif i.get("engine") == "PE":
                    last = None  # unknown PE op: invalidate weight-reuse state
                out.append(i)
            blk["instructions"] = out
    return _json.dumps(d).encode()


import os as _os

if _os.environ.get("KBENCH_LDW_DEDUP") and not getattr(bass_utils, "_ldw_dedup_patched", False):
    _orig_compile_bir_kernel = bass_utils.compile_bir_kernel

    def _compile_bir_kernel_dedup(bir_json, tmpdir, neff_name="file.neff"):
        return _orig_compile_bir_kernel(_dedup_ldweights(bir_json), tmpdir, neff_name)

    bass_utils.compile_bir_kernel = _compile_bir_kernel_dedup
    import concourse.bass2jax as _b2j

    _b2j.compile_bir_kernel = _compile_bir_kernel_dedup
    bass_utils._ldw_dedup_patched = True


def _pack_weights(W1, W2, W3):
    W1 = np.asarray(W1, np.float64)
    W2 = np.asarray(W2, np.float64)
    W3 = np.asarray(W3, np.float64)
    Wcat = np.zeros((5, 128, KOUT, KG), np.float64)
    for p, (i, j) in enumerate(PAIRS):
        c, pp = divmod(p, 128)
        if i < j:
            w3 = W3[i, j] + W3[j, i]   # [k, l]
            w2 = W2[i, j] + W2[j, i]   # [l]
        else:
            w3 = W3[i, i]
            w2 = W2[i, i]
        Wcat[c, pp, :, :D] = w3.T      # col l*34+k
        Wcat[c, pp, :, D] = w2         # out_low column
    for dd in range(D):                # order-1: X rows in chunk 4
        Wcat[4, 16 + dd, :, D] = W1[dd]
    return Wcat.reshape(5, 128, NCOL).astype(np.float32).astype(BF16)


def _build_module():
    nc = bacc.Bacc("TRN2", target_bir_lowering=False, debug=False,
                   enable_asserts=False)
    XBd = nc.dram_tensor("XB", [BLOC, XW], BF, kind="ExternalInput").ap()
    XTd = nc.dram_tensor("XT", [D, BLOC], BF, kind="ExternalInput").ap()
    XEd = nc.dram_tensor("XE", [5, 128, BLOC], BF, kind="ExternalInput").ap()
    XRd = nc.dram_tensor("XR", [5, 128, BLOC], BF, kind="ExternalInput").ap()
    WCd = nc.dram_tensor("WCAT", [5, 128, NCOL], BF, kind="ExternalInput").ap()
    OUTd = nc.dram_tensor("OUT", [BLOC, KOUT], F32, kind="ExternalOutput").ap()

    with ExitStack() as ctx:
        tc = ctx.enter_context(tile.TileContext(nc))
        consts = ctx.enter_context(tc.tile_pool(name="consts", bufs=1))
        xbpool = ctx.enter_context(tc.tile_pool(name="xbpool", bufs=3 * SUPER))
        xepool = ctx.enter_context(tc.tile_pool(name="xepool", bufs=3))
        zpool = ctx.enter_context(tc.tile_pool(name="zpool", bufs=3))
        spool = ctx.enter_context(tc.tile_pool(name="spool", bufs=3))
        upool = ctx.enter_context(tc.tile_pool(name="upool", bufs=3))
        opool = ctx.enter_context(tc.tile_pool(name="opool", bufs=4))
        t3ps = ctx.enter_context(tc.tile_pool(name="t3ps", bufs=2, space="PSUM"))

        w_sb = []
        for c in range(5):
            w = consts.tile([128, NCOL], BF, tag=f"w_{c}")
            nc.scalar.dma_start(out=w, in_=WCd[c])
            w_sb.append(w)

        def build(s):
            """DMA x tiles and build Z^T chunks for supertile s."""
            row0 = s * SP
            xbs = []
            for t in range(SUPER):
                xb = xbpool.tile([P, XW], BF, tag="xb")
                nc.sync.dma_start(out=xb, in_=XBd[row0 + t * P: row0 + (t + 1) * P, :])
                xbs.append(xb)
            zs = []
            for c, pc in enumerate(CHUNKS):
                xe = xepool.tile([pc, SP], BF, tag=f"xe{c}")
                nc.sync.dma_start(out=xe, in_=XEd[c, :pc, row0: row0 + SP])
                xr = xepool.tile([pc, SP], BF, tag=f"xr{c}")
                nc.sync.dma_start(out=xr, in_=XRd[c, :pc, row0: row0 + SP])
                z = zpool.tile([CHUNK_P[c], SP], BF, tag=f"z{c}")
                nc.vector.tensor_mul(z[:pc], xe, xr)
                zs.append(z)
            # order-1 rows: X^T itself sits at partitions 16:48 of chunk 4
            nc.sync.dma_start(out=zs[4][16:48, :], in_=XTd[:, row0: row0 + SP])
            return xbs, zs

        state = build(0)
        for s in range(NSUPER):
            xbs, zs = state
            if s + 1 < NSUPER:
                state = build(s + 1)
            row0 = s * SP
            for t in range(SUPER):
                bsl = slice(t * P, (t + 1) * P)
                t3 = t3ps.tile([P, NCOL], F32, tag="t3")
                for c in range(5):
                    pcp = CHUNK_P[c]
                    first, last = c == 0, c == 4
                    for n0, n1 in ((0, 512), (512, 1024), (1024, NCOL)):
                        nc.tensor.matmul(t3[:, n0:n1], zs[c][:pcp, bsl],
                                         w_sb[c][:pcp, n0:n1],
                                         start=first, stop=last)
                staged = spool.tile([P, NCOL], BF, tag="staged")
                nc.scalar.copy(out=staged, in_=t3)
                u = upool.tile([P, NCOL], BF, tag="u")
                xk = xbs[t][:, :].unsqueeze(1).broadcast_to([P, KOUT, XW])
                nc.vector.tensor_mul(
                    u[:, :].rearrange("p (l k) -> p l k", k=KG),
                    staged[:, :].rearrange("p (l k) -> p l k", k=KG),
                    xk,
                )
                osb = opool.tile([P, KOUT], F32, tag="osb")
                nc.vector.reduce_sum(
                    out=osb, in_=u[:, :].rearrange("p (l k) -> p l k", k=KG),
                    axis=mybir.AxisListType.X,
                )
                nc.scalar.dma_start(out=OUTd[row0 + t * P: row0 + (t + 1) * P, :],
                                    in_=osb)
    nc.compile()
    return nc


_CACHE = {}


def _get_module():
    if "nc" not in _CACHE:
        _CACHE["nc"] = _build_module()
    return _CACHE["nc"]


def kernel(X, W1, W2, W3, bias):
    X = np.ascontiguousarray(np.asarray(X, np.float32))
    bias = np.asarray(bias, np.float32)
    Wcat = _pack_weights(W1, W2, W3)

    nc = _get_module()
    Xb = X.astype(BF16)                      # [B, D] bf16 (single rounding point)
    XbT = np.ascontiguousarray(Xb.T)         # [D, B] bf16
    npad = 5 * 128 - NPAIRS
    XE = np.concatenate([XbT[I_P], np.zeros((npad, B), BF16)], 0).reshape(5, 128, B)
    XR = np.concatenate([XbT[J_P], np.zeros((npad, B), BF16)], 0).reshape(5, 128, B)
    Xpad = np.zeros((B, XW), BF16)
    Xpad[:, :D] = Xb
    Xpad[:, D] = BF16(1.0)
    shards = Xpad.reshape(NCORES, BLOC, XW)
    in_maps = [
        {
            "XB": np.ascontiguousarray(shards[c]),
            "XT": np.ascontiguousarray(XbT[:, c * BLOC:(c + 1) * BLOC]),
            "XE": np.ascontiguousarray(XE[:, :, c * BLOC:(c + 1) * BLOC]),
            "XR": np.ascontiguousarray(XR[:, :, c * BLOC:(c + 1) * BLOC]),
            "WCAT": Wcat,
        }
        for c in range(NCORES)
    ]
    res = bass_utils.run_bass_kernel_spmd(nc, in_maps, core_ids=list(range(NCORES)))
    _CACHE["last_results"] = res
    out = np.concatenate([np.asarray(res.results[c]["OUT"]) for c in range(NCORES)], 0)
    return (out + bias.reshape(1, KOUT)).astype(np.float32)



# revision 2
# speedup vs baseline: 1.0731x; 1.0731x over previous
"""Trainium2 Bass kernel for NeuralTensorLayer (order-1/2/3 polynomial layer).

    out[b,l] = bias[l] + sum_i X[b,i] W1[i,l]
             + sum_ij X[b,i] X[b,j] W2[i,j,l]
             + sum_ijk X[b,i] X[b,j] X[b,k] W3[i,j,k,l]

with B=32768, D=K=32, data-parallel over 8 NeuronCores (4096 rows each).

Strategy (per core):
  * (i,j) symmetry: 528 pairs i<=j against host-symmetrized weights,
    SORTED BY j.  Because X_i X_j X_k is fully symmetric, only monomials
    i<=j<=k are needed: W3m[p,k,l] (perm-symmetrized) is ZERO for k<j(p).
    With pairs sorted by j, the pairs feeding output column-group k form a
    PREFIX of the partition layout, so contraction chunks 1-4 only stream
    the k-SUFFIX of the (l,k) grid:
      chunk kmin:  [0, 15, 22, 27, 31] -> window widths [32, 17, 10, 5, 1]
    PE streaming drops from 5*1152 to ~2340 cycles per 128-row tile.
  * Grid layout (l-major, KG=36): col = l*36 + k', k'=0 holds the low-order
    sum (order-1+2), k'=1..3 zero pads (tree-reduce alignment), k'=4+k the
    order-3 T3 columns.  PSUM tile [128,1152] fp32; all window matmuls land
    within a single 2KB bank (l 0..13 / 14..27 / 28..31 splits).
  * Low-order terms accumulate in a separate [128,32] PSUM bank
    (W2s rows + W1 on the 32 X^T rows of chunk 4); ScalarE merges them into
    the staged tile at k'=0, where the post-multiply by xk[0]=1.0 and the
    tree reduce pick them up for free.
  * Z^T pair products (fp16, from host-pregathered XE/XR) are built on
    GpSimd, keeping VectorE free for the post chain:
      U = staged * Xext (2x), halves-add 36->18 (2x), reduce_sum 18 (1x).
  * fp16 everywhere (PE fp16 == bf16 speed, 8x finer mantissa).
"""

import numpy as np
from contextlib import ExitStack
from itertools import permutations

import concourse.bass as bass
import concourse.bacc as bacc
import concourse.tile as tile
from concourse import mybir
from concourse import bass_utils

F16NP = np.float16

B, D, KOUT = 32768, 32, 32
NCORES = 8
BLOC = B // NCORES          # 4096 rows per core
P = 128                     # rows per tile
SUPER = 4                   # tiles per supertile
SP = SUPER * P              # 512
NSUPER = BLOC // SP         # 8
NPAIRS = D * (D + 1) // 2   # 528
CHUNKS = [128, 128, 128, 128, 16]   # pair rows per contraction chunk
CHUNK_P = [128, 128, 128, 128, 48]  # partitions per chunk (chunk4: +32 X rows)
KMIN = [0, 15, 22, 27, 31]          # k-window lower bound per chunk (j-sorted)
WID = [D - k for k in KMIN]         # [32, 17, 10, 5, 1]
KG = 36                     # grid width: low + 3 pads + 32 k columns
NCOL = KOUT * KG            # 1152 psum grid columns
LSPLIT = [(0, 14), (14, 28), (28, 32)]   # l-ranges per PSUM bank (window MMs)
# weight blocks packed into one [128, WTOT] tensor: per-chunk grid + low
OG = [0]
for c in range(5):
    OG.append(OG[-1] + (NCOL if c == 0 else KOUT * WID[c]))
OL = [OG[5] + 32 * c for c in range(5)]
WTOT = OL[4] + 32

PAIRS = [(i, j) for j in range(D) for i in range(j + 1)]  # sorted by j
I_P = np.array([p[0] for p in PAIRS], np.int32)
J_P = np.array([p[1] for p in PAIRS], np.int32)

F32 = mybir.dt.float32
F16 = mybir.dt.float16

ZBUILD_GPSIMD = True


# Drop redundant LDWEIGHTS from the BIR before walrus codegen: matmuls that
# share a stationary operand (grid bank-splits + low per contraction chunk)
# each carry their own Ldweights (walrus's ldw-opt pass is disabled/broken).
# A load is elided when the previous PE weight-op in SCHEDULED order has a
# byte-identical weight AP and the load itself carries no semaphore
# waits/updates (so the PE weight registers provably still hold the same
# data and no sync edge is lost).
def _dedup_ldweights(bir_json: bytes) -> bytes:
    import json as _json

    d = _json.loads(bir_json)
    for fn in d.get("functions", []):
        for blk in fn.get("blocks", []):
            out = []
            last = None
            for i in blk.get("instructions", []):
                if i.get("engine") == "PE" and i.get("opcode") in ("Ldweights", "Matmult"):
                    w = i["ins"][-1] if i["opcode"] == "Matmult" else i["ins"][0]
                    key = (w.get("memref"), w.get("offset"), _json.dumps(w.get("ap")),
                           w.get("dtype"), _json.dumps(i.get("tile_position")),
                           _json.dumps(i.get("tile_size")), i.get("perf_mode"))
                    if i["opcode"] == "Ldweights":
                        si = i.get("sync_info") or {}
                        if (key == last and not si.get("on_wait")
                                and not si.get("on_update")):
                            continue
                        last = key
                    else:
                        # a Matmult's weight ref mirrors the loaded state
                        # (self-loading or not), so it may refresh `last`
                        last = key
                elif i.get("engine") == "PE":
                    last = None  # unknown PE op: invalidate weight-reuse state
                out.append(i)
            blk["instructions"] = out
    return _json.dumps(d).encode()


if not getattr(bass_utils, "_ldw_dedup_patched", False):
    _orig_compile_bir_kernel = bass_utils.compile_bir_kernel

    def _compile_bir_kernel_dedup(bir_json, tmpdir, neff_name="file.neff"):
        return _orig_compile_bir_kernel(_dedup_ldweights(bir_json), tmpdir, neff_name)

    bass_utils.compile_bir_kernel = _compile_bir_kernel_dedup
    import concourse.bass2jax as _b2j

    _b2j.compile_bir_kernel = _compile_bir_kernel_dedup
    bass_utils._ldw_dedup_patched = True


def _pack_weights(W1, W2, W3):
    W1 = np.asarray(W1, np.float64)
    W2 = np.asarray(W2, np.float64)
    W3 = np.asarray(W3, np.float64)
    # perm-symmetrized monomial weights: W3m[p,k,l], zero for k < j(p)
    W3m = np.zeros((NPAIRS, D, KOUT))
    W2s = np.zeros((NPAIRS, KOUT))
    for p, (i, j) in enumerate(PAIRS):
        W2s[p] = W2[i, j] + W2[j, i] if i < j else W2[i, i]
        for k in range(j, D):
            for perm in set(permutations((i, j, k))):
                W3m[p, k] += W3[perm]
    Wcat = np.zeros((128, WTOT))
    # chunk 0: full grid [128, 1152]; col = l*36 + 4 + k
    g0 = np.zeros((128, KOUT, KG))
    g0[:, :, 4:] = W3m[0:128].transpose(0, 2, 1)   # [p, l, k]
    Wcat[:, OG[0]:OG[1]] = g0.reshape(128, NCOL)
    # chunks 1-4: suffix window [pcp, 32*WID]; col = l*WID + (k - kmin)
    for c in range(1, 5):
        r0, pc = 128 * c, CHUNKS[c]
        blk = W3m[r0:r0 + pc, KMIN[c]:, :].transpose(0, 2, 1)  # [p, l, kw]
        Wcat[:pc, OG[c]:OG[c + 1]] = blk.reshape(pc, KOUT * WID[c])
    # low blocks: W2s on pair rows; W1 on the X rows of chunk 4
    for c in range(5):
        r0, pc = 128 * c, CHUNKS[c]
        Wcat[:pc, OL[c]:OL[c] + 32] = W2s[r0:r0 + pc]
    Wcat[16:48, OL[4]:OL[4] + 32] = W1          # chunk4 partitions 16:48 = X^T
    return Wcat.astype(np.float32).astype(F16NP)


def _build_module():
    nc = bacc.Bacc("TRN2", target_bir_lowering=False, debug=False,
                   enable_asserts=False)
    XBd = nc.dram_tensor("XB", [BLOC, KG], F16, kind="ExternalInput").ap()
    XTd = nc.dram_tensor("XT", [D, BLOC], F16, kind="ExternalInput").ap()
    XEd = nc.dram_tensor("XE", [5, 128, BLOC], F16, kind="ExternalInput").ap()
    XRd = nc.dram_tensor("XR", [5, 128, BLOC], F16, kind="ExternalInput").ap()
    WCd = nc.dram_tensor("WCAT", [128, WTOT], F16, kind="ExternalInput").ap()
    OUTd = nc.dram_tensor("OUT", [BLOC, KOUT], F32, kind="ExternalOutput").ap()

    with ExitStack() as ctx:
        tc = ctx.enter_context(tile.TileContext(nc))
        consts = ctx.enter_context(tc.tile_pool(name="consts", bufs=1))
        xbpool = ctx.enter_context(tc.tile_pool(name="xbpool", bufs=3 * SUPER))
        xepool = ctx.enter_context(tc.tile_pool(name="xepool", bufs=3))
        zpool = ctx.enter_context(tc.tile_pool(name="zpool", bufs=3))
        spool = ctx.enter_context(tc.tile_pool(name="spool", bufs=3))
        upool = ctx.enter_context(tc.tile_pool(name="upool", bufs=3))
        apool = ctx.enter_context(tc.tile_pool(name="apool", bufs=3))
        opool = ctx.enter_context(tc.tile_pool(name="opool", bufs=4))
        t3ps = ctx.enter_context(tc.tile_pool(name="t3ps", bufs=2, space="PSUM"))
        lowps = ctx.enter_context(tc.tile_pool(name="lowps", bufs=2, space="PSUM"))

        w_sb = consts.tile([128, WTOT], F16, tag="wcat")
        nc.scalar.dma_start(out=w_sb, in_=WCd)

        def build(s):
            """DMA x tiles and build Z^T chunks for supertile s."""
            row0 = s * SP
            xbs = []
            for t in range(SUPER):
                xb = xbpool.tile([P, KG], F16, tag="xb")
                nc.sync.dma_start(out=xb, in_=XBd[row0 + t * P: row0 + (t + 1) * P, :])
                xbs.append(xb)
            zs = []
            for c, pc in enumerate(CHUNKS):
                xe = xepool.tile([pc, SP], F16, tag=f"xe{c}")
                nc.sync.dma_start(out=xe, in_=XEd[c, :pc, row0: row0 + SP])
                xr = xepool.tile([pc, SP], F16, tag=f"xr{c}")
                nc.sync.dma_start(out=xr, in_=XRd[c, :pc, row0: row0 + SP])
                z = zpool.tile([CHUNK_P[c], SP], F16, tag=f"z{c}")
                if ZBUILD_GPSIMD:
                    nc.gpsimd.tensor_mul(z[:pc], xe, xr)
                else:
                    nc.vector.tensor_mul(z[:pc], xe, xr)
                zs.append(z)
            # order-1 rows: X^T itself sits at partitions 16:48 of chunk 4
            nc.sync.dma_start(out=zs[4][16:48, :], in_=XTd[:, row0: row0 + SP])
            return xbs, zs

        state = build(0)
        for s in range(NSUPER):
            xbs, zs = state
            if s + 1 < NSUPER:
                state = build(s + 1)
            row0 = s * SP
            for t in range(SUPER):
                bsl = slice(t * P, (t + 1) * P)
                t3 = t3ps.tile([P, NCOL], F32, tag="t3")
                plow = lowps.tile([P, KOUT], F32, tag="plow")
                t3v = t3[:, :].rearrange("p (l k) -> p l k", k=KG)
                for c in range(5):
                    pcp = CHUNK_P[c]
                    first, last = c == 0, c == 4
                    zc = zs[c][:pcp, bsl]
                    if c == 0:
                        for n0, n1 in ((0, 512), (512, 1024), (1024, NCOL)):
                            nc.tensor.matmul(t3[:, n0:n1], zc,
                                             w_sb[:pcp, n0:n1],
                                             start=True, stop=False)
                    else:
                        wv = w_sb[:pcp, OG[c]:OG[c + 1]].rearrange(
                            "p (l k) -> p l k", k=WID[c])
                        for l0, l1 in LSPLIT:
                            nc.tensor.matmul(
                                t3v[:, l0:l1, KMIN[c] + 4:KG],
                                zc, wv[:, l0:l1, :],
                                start=False, stop=last)
                    nc.tensor.matmul(plow[:, :], zc,
                                     w_sb[:pcp, OL[c]:OL[c] + 32],
                                     start=first, stop=last)
                staged = spool.tile([P, NCOL], F16, tag="staged")
                nc.scalar.copy(out=staged, in_=t3)
                sv = staged[:, :].rearrange("p (l k) -> p l k", k=KG)
                nc.scalar.copy(out=sv[:, :, 0], in_=plow[:, :])
                u = upool.tile([P, NCOL], F16, tag="u")
                uv = u[:, :].rearrange("p (l k) -> p l k", k=KG)
                xk = xbs[t][:, :].unsqueeze(1).broadcast_to([P, KOUT, KG])
                nc.vector.tensor_mul(uv, sv, xk)
                a = apool.tile([P, KOUT * 18], F16, tag="a")
                av = a[:, :].rearrange("p (l k) -> p l k", k=18)
                nc.vector.tensor_add(av, uv[:, :, 0:18], uv[:, :, 18:KG])
                osb = opool.tile([P, KOUT], F32, tag="osb")
                nc.vector.reduce_sum(out=osb, in_=av,
                                     axis=mybir.AxisListType.X)
                nc.scalar.dma_start(out=OUTd[row0 + t * P: row0 + (t + 1) * P, :],
                                    in_=osb)
    nc.compile()
    return nc


_CACHE = {}


def _get_module():
    if "nc" not in _CACHE:
        _CACHE["nc"] = _build_module()
    return _CACHE["nc"]


def kernel(X, W1, W2, W3, bias):
    X = np.ascontiguousarray(np.asarray(X, np.float32))
    bias = np.asarray(bias, np.float32)
    Wcat = _pack_weights(W1, W2, W3)

    nc = _get_module()
    Xh = X.astype(F16NP)                     # [B, D] fp16 (single rounding point)
    XhT = np.ascontiguousarray(Xh.T)         # [D, B] fp16
    npad = 5 * 128 - NPAIRS
    XE = np.concatenate([XhT[I_P], np.zeros((npad, B), F16NP)], 0).reshape(5, 128, B)
    XR = np.concatenate([XhT[J_P], np.zeros((npad, B), F16NP)], 0).reshape(5, 128, B)
    Xpad = np.zeros((B, KG), F16NP)
    Xpad[:, 0] = F16NP(1.0)                  # low column passes through
    Xpad[:, 4:] = Xh
    shards = Xpad.reshape(NCORES, BLOC, KG)
    in_maps = [
        {
            "XB": np.ascontiguousarray(shards[c]),
            "XT": np.ascontiguousarray(XhT[:, c * BLOC:(c + 1) * BLOC]),
            "XE": np.ascontiguousarray(XE[:, :, c * BLOC:(c + 1) * BLOC]),
            "XR": np.ascontiguousarray(XR[:, :, c * BLOC:(c + 1) * BLOC]),
            "WCAT": Wcat,
        }
        for c in range(NCORES)
    ]
    res = bass_utils.run_bass_kernel_spmd(nc, in_maps, core_ids=list(range(NCORES)))
    _CACHE["last_results"] = res
    out = np.concatenate([np.asarray(res.results[c]["OUT"]) for c in range(NCORES)], 0)
    return (out + bias.reshape(1, KOUT)).astype(np.float32)


# revision 3
# speedup vs baseline: 1.2763x; 1.1893x over previous
"""Trainium2 Bass kernel for NeuralTensorLayer (order-1/2/3 polynomial layer).

    out[b,l] = bias[l] + sum_i X[b,i] W1[i,l]
             + sum_ij X[b,i] X[b,j] W2[i,j,l]
             + sum_ijk X[b,i] X[b,j] X[b,k] W3[i,j,k,l]

with B=32768, D=K=32, data-parallel over 8 NeuronCores (4096 rows each).

Strategy (per core):
  * (i,j) symmetry: 528 pairs i<=j against host-symmetrized weights,
    SORTED BY j.  Because X_i X_j X_k is fully symmetric, only monomials
    i<=j<=k are needed: W3m[p,k,l] (perm-symmetrized) is ZERO for k<j(p).
    With pairs sorted by j, the pairs feeding output column-group k form a
    PREFIX of the partition layout, so contraction chunks 1-4 only stream
    the k-SUFFIX of the (l,k) grid:
      chunk kmin:  [0, 15, 22, 27, 31] -> window widths [32, 17, 10, 5, 1]
    PE streaming drops from 5*1024 to ~2240 cycles per 128-row tile.
  * Grid layout (l-major, KG=32): col = l*32 + k; PSUM tile [128, 1056]
    fp32 where cols 1024:1056 hold the low-order sums (W2s rows + W1 on the
    32 X^T rows of chunk 4).  Banks align exactly: window matmuls split at
    l=16; low matmuls are [*, 32] into bank 2 of the same tile (keeps each
    chunk's matmuls adjacent so redundant LDWEIGHTS dedup to one per chunk).
  * Per tile: ScalarE stages grid (fp32->fp16) and low separately; VectorE
    post runs SUPERTILE-BATCHED (4 tiles per op, amortizing the ~60-cycle
    DVE op overhead): U = staged*X (2x), tree halves-add 32->16->8 (2x),
    reduce_sum 8 (1x), + low add.  Z^T pair products on VectorE too
    (GpSimd shares the DVE SBUF port - measured contention tax ~40%).
  * fp16 everywhere (PE fp16 == bf16 speed, 8x finer mantissa).
"""

import numpy as np
from contextlib import ExitStack
from itertools import permutations

import concourse.bass as bass
import concourse.bacc as bacc
import concourse.tile as tile
from concourse import mybir
from concourse import bass_utils

F16NP = np.float16

B, D, KOUT = 32768, 32, 32
NCORES = 8
BLOC = B // NCORES          # 4096 rows per core
P = 128                     # rows per tile
SUPER = 4                   # tiles per supertile
SP = SUPER * P              # 512
NSUPER = BLOC // SP         # 8
NPAIRS = D * (D + 1) // 2   # 528
CHUNKS = [128, 128, 128, 128, 16]   # pair rows per contraction chunk
CHUNK_P = [128, 128, 128, 128, 48]  # partitions per chunk (chunk4: +32 X rows)
KMIN = [0, 15, 22, 27, 31]          # k-window lower bound per chunk (j-sorted)
WID = [D - k for k in KMIN]         # [32, 17, 10, 5, 1]
KG = 32                     # grid width per l
NCOL = KOUT * KG            # 1024 grid columns
NCOLT = NCOL + 32           # + low block
LSPLIT = [(0, 16), (16, 32)]        # l-ranges per PSUM bank (window MMs)
# weight blocks packed into one [128, WTOT] tensor: per-chunk grid + low
OG = [0]
for c in range(5):
    OG.append(OG[-1] + (NCOL if c == 0 else KOUT * WID[c]))
OL = [OG[5] + 32 * c for c in range(5)]
WTOT = OL[4] + 32

PAIRS = [(i, j) for j in range(D) for i in range(j + 1)]  # sorted by j
I_P = np.array([p[0] for p in PAIRS], np.int32)
J_P = np.array([p[1] for p in PAIRS], np.int32)

F32 = mybir.dt.float32
F16 = mybir.dt.float16


# Drop redundant LDWEIGHTS from the BIR before walrus codegen: matmuls that
# share a stationary operand (grid bank-splits + low per contraction chunk)
# each carry their own Ldweights (walrus's ldw-opt pass is disabled/broken).
# A load is elided when the previous PE weight-op in SCHEDULED order has a
# byte-identical weight AP and the load itself carries no semaphore
# waits/updates (so the PE weight registers provably still hold the same
# data and no sync edge is lost).
def _dedup_ldweights(bir_json: bytes) -> bytes:
    import json as _json

    d = _json.loads(bir_json)
    for fn in d.get("functions", []):
        for blk in fn.get("blocks", []):
            out = []
            last = None
            for i in blk.get("instructions", []):
                if i.get("engine") == "PE" and i.get("opcode") in ("Ldweights", "Matmult"):
                    w = i["ins"][-1] if i["opcode"] == "Matmult" else i["ins"][0]
                    key = (w.get("memref"), w.get("offset"), _json.dumps(w.get("ap")),
                           w.get("dtype"), _json.dumps(i.get("tile_position")),
                           _json.dumps(i.get("tile_size")), i.get("perf_mode"))
                    if i["opcode"] == "Ldweights":
                        si = i.get("sync_info") or {}
                        if (key == last and not si.get("on_wait")
                                and not si.get("on_update")):
                            continue
                        last = key
                    else:
                        # a Matmult's weight ref mirrors the loaded state
                        # (self-loading or not), so it may refresh `last`
                        last = key
                elif i.get("engine") == "PE":
                    last = None  # unknown PE op: invalidate weight-reuse state
                out.append(i)
            blk["instructions"] = out
    return _json.dumps(d).encode()


if not getattr(bass_utils, "_ldw_dedup_patched", False):
    _orig_compile_bir_kernel = bass_utils.compile_bir_kernel

    def _compile_bir_kernel_dedup(bir_json, tmpdir, neff_name="file.neff"):
        return _orig_compile_bir_kernel(_dedup_ldweights(bir_json), tmpdir, neff_name)

    bass_utils.compile_bir_kernel = _compile_bir_kernel_dedup
    import concourse.bass2jax as _b2j

    _b2j.compile_bir_kernel = _compile_bir_kernel_dedup
    bass_utils._ldw_dedup_patched = True


def _pack_weights(W1, W2, W3):
    W1 = np.asarray(W1, np.float64)
    W2 = np.asarray(W2, np.float64)
    W3 = np.asarray(W3, np.float64)
    # perm-symmetrized monomial weights: W3m[p,k,l], zero for k < j(p)
    W3m = np.zeros((NPAIRS, D, KOUT))
    W2s = np.zeros((NPAIRS, KOUT))
    for p, (i, j) in enumerate(PAIRS):
        W2s[p] = W2[i, j] + W2[j, i] if i < j else W2[i, i]
        for k in range(j, D):
            for perm in set(permutations((i, j, k))):
                W3m[p, k] += W3[perm]
    Wcat = np.zeros((128, WTOT))
    # chunk 0: full grid [128, 1024]; col = l*32 + k
    Wcat[:, OG[0]:OG[1]] = W3m[0:128].transpose(0, 2, 1).reshape(128, NCOL)
    # chunks 1-4: suffix window [pcp, 32*WID]; col = l*WID + (k - kmin)
    for c in range(1, 5):
        r0, pc = 128 * c, CHUNKS[c]
        blk = W3m[r0:r0 + pc, KMIN[c]:, :].transpose(0, 2, 1)  # [p, l, kw]
        Wcat[:pc, OG[c]:OG[c + 1]] = blk.reshape(pc, KOUT * WID[c])
    # low blocks: W2s on pair rows; W1 on the X rows of chunk 4
    for c in range(5):
        r0, pc = 128 * c, CHUNKS[c]
        Wcat[:pc, OL[c]:OL[c] + 32] = W2s[r0:r0 + pc]
    Wcat[16:48, OL[4]:OL[4] + 32] = W1          # chunk4 partitions 16:48 = X^T
    return Wcat.astype(np.float32).astype(F16NP)


def _build_module():
    nc = bacc.Bacc("TRN2", target_bir_lowering=False, debug=False,
                   enable_asserts=False)
    # XS: per-supertile X blocks, partition-interleaved [s][p][t][k]
    XSd = nc.dram_tensor("XS", [NSUPER, P, SUPER * D], F16, kind="ExternalInput").ap()
    XTd = nc.dram_tensor("XT", [D, BLOC], F16, kind="ExternalInput").ap()
    # chunks 0-3 pair operands packed for a single merged Z multiply
    XEd = nc.dram_tensor("XE", [128, 4, BLOC], F16, kind="ExternalInput").ap()
    XRd = nc.dram_tensor("XR", [128, 4, BLOC], F16, kind="ExternalInput").ap()
    XE4d = nc.dram_tensor("XE4", [16, BLOC], F16, kind="ExternalInput").ap()
    XR4d = nc.dram_tensor("XR4", [16, BLOC], F16, kind="ExternalInput").ap()
    WCd = nc.dram_tensor("WCAT", [128, WTOT], F16, kind="ExternalInput").ap()
    OUTd = nc.dram_tensor("OUT", [BLOC, KOUT], F32, kind="ExternalOutput").ap()

    with ExitStack() as ctx:
        tc = ctx.enter_context(tile.TileContext(nc))
        consts = ctx.enter_context(tc.tile_pool(name="consts", bufs=1))
        xspool = ctx.enter_context(tc.tile_pool(name="xspool", bufs=3))
        xepool = ctx.enter_context(tc.tile_pool(name="xepool", bufs=3))
        zpool = ctx.enter_context(tc.tile_pool(name="zpool", bufs=3))
        spool = ctx.enter_context(tc.tile_pool(name="spool", bufs=2))
        slpool = ctx.enter_context(tc.tile_pool(name="slpool", bufs=2))
        upool = ctx.enter_context(tc.tile_pool(name="upool", bufs=2))
        apool = ctx.enter_context(tc.tile_pool(name="apool", bufs=2))
        opool = ctx.enter_context(tc.tile_pool(name="opool", bufs=2))
        t3ps = ctx.enter_context(tc.tile_pool(name="t3ps", bufs=2, space="PSUM"))

        w_sb = consts.tile([128, WTOT], F16, tag="wcat")
        nc.scalar.dma_start(out=w_sb, in_=WCd)

        def build(s):
            """DMA x tiles and build Z^T chunks for supertile s."""
            row0 = s * SP
            xs = xspool.tile([P, SUPER * D], F16, tag="xs")
            nc.sync.dma_start(out=xs, in_=XSd[s])
            xe = xepool.tile([128, 4 * SP], F16, tag="xe")
            nc.sync.dma_start(out=xe[:, :].rearrange("p (c b) -> p c b", c=4),
                              in_=XEd[:, :, row0: row0 + SP])
            xr = xepool.tile([128, 4 * SP], F16, tag="xr")
            nc.sync.dma_start(out=xr[:, :].rearrange("p (c b) -> p c b", c=4),
                              in_=XRd[:, :, row0: row0 + SP])
            xe4 = xepool.tile([16, SP], F16, tag="xe4")
            nc.sync.dma_start(out=xe4, in_=XE4d[:, row0: row0 + SP])
            xr4 = xepool.tile([16, SP], F16, tag="xr4")
            nc.sync.dma_start(out=xr4, in_=XR4d[:, row0: row0 + SP])
            z = zpool.tile([128, 4 * SP], F16, tag="z")
            nc.vector.tensor_mul(z, xe, xr)
            z4 = zpool.tile([48, SP], F16, tag="z4")
            nc.vector.tensor_mul(z4[:16, :], xe4, xr4)
            # order-1 rows: X^T itself sits at partitions 16:48 of chunk 4
            nc.sync.dma_start(out=z4[16:48, :], in_=XTd[:, row0: row0 + SP])
            zv = z[:, :].rearrange("p (c b) -> p c b", c=4)
            zs = [zv[:, c, :] for c in range(4)] + [z4[:, :]]
            return xs, zs

        state = build(0)
        for s in range(NSUPER):
            xs, zs = state
            if s + 1 < NSUPER:
                state = build(s + 1)
            row0 = s * SP
            staged = spool.tile([P, SUPER * NCOL], F16, tag="staged")
            slow = slpool.tile([P, SUPER * 32], F16, tag="slow")
            for t in range(SUPER):
                bsl = slice(t * P, (t + 1) * P)
                t3 = t3ps.tile([P, NCOLT], F32, tag="t3")
                t3v = t3[:, :NCOL].rearrange("p (l k) -> p l k", k=KG)
                for c in range(5):
                    pcp = CHUNK_P[c]
                    first, last = c == 0, c == 4
                    zc = zs[c][:pcp, bsl]
                    if c == 0:
                        for n0, n1 in ((0, 512), (512, 1024)):
                            nc.tensor.matmul(t3[:, n0:n1], zc,
                                             w_sb[:pcp, n0:n1],
                                             start=True, stop=False)
                    else:
                        wv = w_sb[:pcp, OG[c]:OG[c + 1]].rearrange(
                            "p (l k) -> p l k", k=WID[c])
                        for l0, l1 in LSPLIT:
                            nc.tensor.matmul(
                                t3v[:, l0:l1, KMIN[c]:KG],
                                zc, wv[:, l0:l1, :],
                                start=False, stop=last)
                    nc.tensor.matmul(t3[:, NCOL:NCOLT], zc,
                                     w_sb[:pcp, OL[c]:OL[c] + 32],
                                     start=first, stop=last)
                nc.scalar.copy(out=staged[:, t * NCOL:(t + 1) * NCOL],
                               in_=t3[:, :NCOL])
                nc.scalar.copy(out=slow[:, t * 32:(t + 1) * 32],
                               in_=t3[:, NCOL:NCOLT])
            # supertile-batched post: U = staged * X, tree-reduce over k
            u = upool.tile([P, SUPER * NCOL], F16, tag="u")
            uv = u[:, :].rearrange("p (t l k) -> p t l k", t=SUPER, k=KG)
            sv = staged[:, :].rearrange("p (t l k) -> p t l k", t=SUPER, k=KG)
            xk = xs[:, :].rearrange("p (t k) -> p t k", t=SUPER) \
                .unsqueeze(2).broadcast_to([P, SUPER, KOUT, KG])
            nc.vector.tensor_mul(uv, sv, xk)
            a1 = apool.tile([P, SUPER * KOUT * 16], F16, tag="a1")
            a1v = a1[:, :].rearrange("p (g k) -> p g k", k=16)
            uv2 = u[:, :].rearrange("p (g k) -> p g k", k=KG)
            nc.vector.tensor_add(a1v, uv2[:, :, 0:16], uv2[:, :, 16:KG])
            a2 = apool.tile([P, SUPER * KOUT * 8], F16, tag="a2")
            a2v = a2[:, :].rearrange("p (g k) -> p g k", k=8)
            nc.vector.tensor_add(a2v, a1v[:, :, 0:8], a1v[:, :, 8:16])
            osb = opool.tile([P, SUPER * KOUT], F32, tag="osb")
            nc.vector.reduce_sum(out=osb, in_=a2v, axis=mybir.AxisListType.X)
            nc.vector.tensor_add(osb[:, :], osb[:, :], slow[:, :])
            nc.sync.dma_start(
                out=OUTd[row0: row0 + SP, :].rearrange("(t p) k -> p t k", p=P),
                in_=osb[:, :].rearrange("p (t k) -> p t k", t=SUPER))
    nc.compile()
    return nc


_CACHE = {}


def _get_module():
    if "nc" not in _CACHE:
        _CACHE["nc"] = _build_module()
    return _CACHE["nc"]


def kernel(X, W1, W2, W3, bias):
    X = np.ascontiguousarray(np.asarray(X, np.float32))
    bias = np.asarray(bias, np.float32)
    Wcat = _pack_weights(W1, W2, W3)

    nc = _get_module()
    Xh = X.astype(F16NP)                     # [B, D] fp16 (single rounding point)
    XhT = np.ascontiguousarray(Xh.T)         # [D, B] fp16
    XE = XhT[I_P[:512]].reshape(4, 128, B).transpose(1, 0, 2)   # [128, 4, B]
    XR = XhT[J_P[:512]].reshape(4, 128, B).transpose(1, 0, 2)
    XE4 = XhT[I_P[512:]]                     # [16, B]
    XR4 = XhT[J_P[512:]]
    # per-supertile partition-interleaved X blocks [s, p, t, k]
    XS = Xh.reshape(NCORES, NSUPER, SUPER, P, D).transpose(0, 1, 3, 2, 4) \
        .reshape(NCORES, NSUPER, P, SUPER * D)
    in_maps = [
        {
            "XS": np.ascontiguousarray(XS[c]),
            "XT": np.ascontiguousarray(XhT[:, c * BLOC:(c + 1) * BLOC]),
            "XE": np.ascontiguousarray(XE[:, :, c * BLOC:(c + 1) * BLOC]),
            "XR": np.ascontiguousarray(XR[:, :, c * BLOC:(c + 1) * BLOC]),
            "XE4": np.ascontiguousarray(XE4[:, c * BLOC:(c + 1) * BLOC]),
            "XR4": np.ascontiguousarray(XR4[:, c * BLOC:(c + 1) * BLOC]),
            "WCAT": Wcat,
        }
        for c in range(NCORES)
    ]
    res = bass_utils.run_bass_kernel_spmd(nc, in_maps, core_ids=list(range(NCORES)))
    _CACHE["last_results"] = res
    out = np.concatenate([np.asarray(res.results[c]["OUT"]) for c in range(NCORES)], 0)
    return (out + bias.reshape(1, KOUT)).astype(np.float32)
